# revision 15
# baseline (speedup 1.0000x reference)
"""Trainium2 Bass kernel for the DCL contrastive loss (nn_DCL_11776800325979).

Reference computation:
    feats = concat([z1, z2])                       # [8192, 128]
    cos = (feats @ feats.T) / max(|f_i||f_j|, eps) # [8192, 8192]
    cos[i,i] = -9e15 ; s = cos / 0.1
    pos_i = s[i, (i+4096) % 8192]
    neg = s with the pos column also masked
    loss = mean(-pos_i + logsumexp(neg, axis=-1))

Strategy (8 NeuronCores, data-parallel over rows):
  Each core receives feats rolled by -c*1024 rows (bf16) and computes the
  loss rows for *local* rows 0..1023 against all 8192 columns.  Rolling
  both index spaces by the same amount preserves the self offset (i==j)
  and the positive-pair offset ((i+4096) mod 8192), so one NEFF runs SPMD
  on all 8 cores.

  Preamble (per 2048-row chunk, pipelined): row-major bf16 half-loads on
  both HWDGE rings (contiguous 4KB/partition descriptors), bf16
  square+reduce -> |f|^2, bf16 magic-seed Newton rsqrt (1 iter) ->
  1/|f|, one broadcast-multiply normalizes the rows (chunk 0 on DVE,
  rest on Pool), store to a DRAM scratch + xbar DMA-transpose reload
  (both on the sync HWDGE ring) -> normalized G^T chunks in SBUF.

  Main loop: ACT covers most column tiles with exp(10*cos) [128,1024]
  activations + fused row-sum accumulators.  OFF_J column tiles are
  computed *transposed* (cols as partitions): DVE turns the PSUM logits
  into bf16 Schraudolph exp bits (one tensor_scalar), and the PE
  row-sums them via ones-matmuls accumulated into a [1,1024] PSUM strip
  that is folded back into the row sums at the end.  Self/pos diagonal
  terms are extracted from the ACT bf16 exp output (eyeb multiply on
  Pool, row-reduce on DVE) and subtracted.

  epilogue: loss rows = ln(RSC/posE) via a DVE log bit-trick plus one
  exp-based Newton refinement, so the Exp activation table loaded in the
  preamble is the only table load.  Host averages.
"""

import math

import numpy as np

D = 128          # feature dim (= contraction dim = partitions)
N2 = 8192        # 2N rows
NCORES = 8
RPC = N2 // NCORES          # rows per core = 1024
RB = RPC // 128             # row blocks per core = 8
CG = 4                      # chunks of 2048 (both row-chunks and G-chunks)
CGW = N2 // CG              # chunk width = 2048
TW = 1024                   # logit tile width
NJ = N2 // TW               # 8 column tiles per row block
INV_TEMP = 10.0
MAGIC16 = 0x5F37            # fast inverse sqrt seed (bf16 bits)
LN2 = math.log(2.0)

# bf16 Schraudolph exp: i16 bits = round(cos * SCH_A16 + SCH_B16);
# bitcast bf16 ~ exp(10*cos)
SCH_A16 = (2.0 ** 7) * INV_TEMP / LN2
SCH_B16 = (127.0 - 0.0564) * (2.0 ** 7)
# f32 log bit-trick: ln(x) ~ bits(x) * LOG_K - LOG_C0   (then one Newton)
LOG_K = LN2 / (2.0 ** 23)
LOG_C0 = (127.0 - 0.0430) * LN2

# column tiles computed transposed with PE row-sum accumulation.
# j=0 (self diag) and j=4 (pos diag) must stay on ACT.
OFF_J = (1, 3, 7)

_CACHE = {}
LAST_RESULTS = None


def _build():
    if "nc" in _CACHE:
        return _CACHE["nc"]

    from contextlib import ExitStack

    import concourse.bass as bass  # noqa: F401
    import concourse.mybir as mybir
    import concourse.tile as tile
    from concourse import bacc

    f32 = mybir.dt.float32
    i16 = mybir.dt.int16
    i32 = mybir.dt.int32
    bf16 = mybir.dt.bfloat16
    AF = mybir.ActivationFunctionType
    ALU = mybir.AluOpType
    X = mybir.AxisListType.X

    nc = bacc.Bacc(
        "TRN2",
        target_bir_lowering=False,
        debug=False,
        enable_asserts=False,
        num_devices=NCORES,
    )

    featsb = nc.dram_tensor("featsb", [N2, D], bf16, kind="ExternalInput").ap()
    eyeb_d = nc.dram_tensor("eyeb", [128, 128], bf16, kind="ExternalInput").ap()
    scratch = nc.dram_tensor("scratch", [N2, D], bf16, kind="Internal").ap()
    offd = nc.dram_tensor("offsums", [3, RPC], f32, kind="Internal").ap()
    out_d = nc.dram_tensor("loss_rows", [128, RB], f32, kind="ExternalOutput").ap()

    n_off = len(OFF_J) * RB          # transposed column-block tiles

    with tile.TileContext(nc) as tc, ExitStack() as ctx:
        consts = ctx.enter_context(tc.tile_pool(name="consts", bufs=1))
        gpool = ctx.enter_context(tc.tile_pool(name="G", bufs=1))
        fpool = ctx.enter_context(tc.tile_pool(name="F", bufs=4))
        fnpool = ctx.enter_context(tc.tile_pool(name="FN", bufs=4))
        sqpool = ctx.enter_context(tc.tile_pool(name="SQ", bufs=2))
        scrp = ctx.enter_context(tc.tile_pool(name="scr", bufs=2))
        stat = ctx.enter_context(tc.tile_pool(name="stat", bufs=1))
        epool = ctx.enter_context(tc.tile_pool(name="E", bufs=16))
        ipool = ctx.enter_context(tc.tile_pool(name="I", bufs=4))
        xpool = ctx.enter_context(tc.tile_pool(name="XT", bufs=3))
        ppool = ctx.enter_context(tc.tile_pool(name="P", bufs=3, space="PSUM"))
        opool = ctx.enter_context(tc.tile_pool(name="OP", bufs=1, space="PSUM"))

        def _dep(after, before, reason):
            a = getattr(after, "ins", after)
            b = getattr(before, "ins", before)
            tile.add_dep_helper(a, b, reason=reason)

        eyeb = consts.tile([128, 128], bf16)
        onesb = consts.tile([128, 1], bf16)
        nc.vector.memset(onesb[:], 1.0)
        magicT = consts.tile([128, 16], i16)
        nc.vector.memset(magicT[:], MAGIC16)
        c15 = consts.tile([128, 16], bf16)
        nc.vector.memset(c15[:], 1.5)
        dummy_in = consts.tile([128, 1], f32)
        nc.vector.memset(dummy_in[:], 1.0)
        dummy_out = consts.tile([128, 1], f32)

        SS = stat.tile([128, 64], bf16)      # per-row |f|^2 (col t = row tile)
        RNb = stat.tile([128, 64], bf16)     # 1/|f| (bf16)
        SUMS = stat.tile([128, RB * NJ], f32)   # row sumexp per (b, j), ACT js
        nc.vector.memset(SUMS[:], 0.0)
        SELFE = stat.tile([128, RB], f32)    # exp(self) per row block
        POSE = stat.tile([128, RB], f32)     # exp(pos) per row block

        G = [
            gpool.tile([128, CGW], bf16, tag=f"G{g}", name=f"G{g}")
            for g in range(CG)
        ]

        # ---- preamble: half-chunk loads on the two HWDGE rings ----
        # partition p of chunk g holds rows g*2048 + 16p + t (contiguous 4KB)
        Fgs = []
        with tc.high_priority():
            for g in range(CG):
                Fg = fpool.tile([128, CGW], bf16, tag=f"F{g}", name=f"Fg{g}")
                Fgs.append(Fg)
            for g in range(CG):
                for h, eng in ((0, nc.scalar), (1, nc.sync)):
                    # half h = partitions 64h..64h+63 = rows g*2048+1024h..
                    eng.dma_start(
                        Fgs[g][64 * h:64 * (h + 1), :].rearrange(
                            "p (t d) -> p t d", d=128),
                        featsb[g * CGW + h * TW:g * CGW + (h + 1) * TW, :]
                        .rearrange("(p t) d -> p t d", t=16),
                    )
        nc.gpsimd.dma_start(eyeb[:], eyeb_d[:, :])
        # trigger the activation table load during the preamble; Square
        # then Exp narrows the choice to a set that serves both (6 or 22)
        nc.scalar.activation(dummy_out[:], dummy_in[:], AF.Square)
        nc.scalar.activation(dummy_out[:], dummy_in[:], AF.Exp)

        def rsqrt_chunk(g):
            """RNb[:, g*16:(g+1)*16] = 1/sqrt(SS[..]), bf16, 1 Newton iter."""
            lo, hi = g * 16, (g + 1) * 16
            x = SS[:, lo:hi]
            y = scrp.tile([128, 16], bf16, tag="nw_y", name="nw_y")
            t = scrp.tile([128, 16], bf16, tag="nw_t", name="nw_t")
            nc.vector.tensor_scalar(
                y[:].bitcast(i16), x.bitcast(i16), 1, None,
                op0=ALU.logical_shift_right,
            )
            nc.vector.tensor_sub(y[:].bitcast(i16), magicT[:], y[:].bitcast(i16))
            nc.vector.tensor_mul(t[:], y[:], y[:])
            nc.vector.tensor_mul(t[:], t[:], x)
            nc.vector.scalar_tensor_tensor(
                t[:], t[:], -0.5, c15[:], ALU.mult, ALU.add
            )
            nc.vector.tensor_mul(RNb[:, lo:hi], y[:], t[:])

        # ---- per-chunk: norms -> normalize -> store -> transposed load ----
        for g in range(CG):
            from contextlib import nullcontext
            prio = tc.high_priority() if g == 0 else nullcontext()
            with prio:
                Fg = Fgs[g]
                SQ = sqpool.tile([128, CGW], bf16, tag="SQ", name="SQ")
                nc.scalar.activation(SQ[:], Fg[:], AF.Square)
                with nc.allow_low_precision(
                    reason="|f|^2 in bf16 is plenty for a 1/|f| scale"
                ):
                    nc.vector.reduce_sum(
                        SS[:, g * 16:(g + 1) * 16],
                        SQ[:].rearrange("p (t d) -> p t d", d=128), axis=X,
                    )
                rsqrt_chunk(g)
                Fn = fnpool.tile([128, CGW], bf16, tag=f"FN{g}", name=f"Fn{g}")
                mul_eng = nc.vector if g == 0 else nc.gpsimd
                mul_eng.tensor_mul(
                    Fn[:].rearrange("p (t d) -> p t d", d=128),
                    Fg[:].rearrange("p (t d) -> p t d", d=128),
                    RNb[:, g * 16:(g + 1) * 16, None].to_broadcast((128, 16, 128)),
                )
                if g == 0:
                    # halve the DRAM roundtrip: rows 0..1023 (partitions
                    # 0..63) first so phase A's first tiles start sooner
                    for h in range(2):
                        st_i = nc.sync.dma_start(
                            scratch[h * TW:(h + 1) * TW, :].rearrange(
                                "(p t) d -> p t d", t=16),
                            Fn[64 * h:64 * (h + 1), :].rearrange(
                                "p (t d) -> p t d", d=128),
                        )
                        tl_i = nc.sync.dma_start(
                            G[0][:, h * TW:(h + 1) * TW],
                            scratch[h * TW:(h + 1) * TW, :],
                            transpose=True,
                        )
                        _dep(tl_i, st_i, "transpose load reads scratch half")
                else:
                    st_i = nc.sync.dma_start(
                        scratch[g * CGW:(g + 1) * CGW, :].rearrange(
                            "(p t) d -> p t d", t=16),
                        Fn[:].rearrange("p (t d) -> p t d", d=128),
                    )
                    tl_i = nc.sync.dma_start(
                        G[g][:], scratch[g * CGW:(g + 1) * CGW, :],
                        transpose=True,
                    )
                    _dep(tl_i, st_i, "transpose load reads scratch chunk")

        # ---- main loop ----
        accum_insts = []
        extract_insts = []
        OFFP = opool.tile([128, TW], f32, tag="OFFP", name="OFFP")
        off_cnt = [0]
        ones_mms = []
        off_stores = []

        def row_tile(b, j):
            P = ppool.tile([128, TW], f32, tag="P", name="P")
            for t in range(2):
                off = (j % 2) * TW + t * 512
                nc.tensor.matmul(
                    P[:, t * 512:(t + 1) * 512],
                    G[0][:, b * 128:(b + 1) * 128],
                    G[j // 2][:, off:off + 512],
                    start=True,
                    stop=True,
                )
            k = b * NJ + j
            E = epool.tile([128, TW], bf16, tag="E", name="E")
            act_i = nc.scalar.activation(
                E[:], P[:], AF.Exp, scale=INV_TEMP,
                accum_out=SUMS[:, k:k + 1],
            )
            accum_insts.append(act_i)
            if j == 0 or j == 4:
                # diagonal of this block holds exp(self)/exp(pos)
                dst = SELFE if j == 0 else POSE
                xr = xpool.tile([128, 128], bf16, tag="xs", name="xs")
                nc.gpsimd.tensor_mul(
                    xr[:], E[:, b * 128:(b + 1) * 128], eyeb[:]
                )
                e2 = nc.vector.reduce_sum(dst[:, b:b + 1], xr[:], axis=X)
                extract_insts.append(e2)

        def trans_tile(cb):
            """Column block cb (128 cols) vs all 1024 core rows, transposed.
            exp via bf16 Schraudolph bits on DVE; row sums via PE
            ones-matmuls accumulated into OFFP[0, :]."""
            g, off = cb // 16, (cb % 16) * 128
            Pt = ppool.tile([128, TW], f32, tag="P", name="Pt")
            for t in range(2):
                nc.tensor.matmul(
                    Pt[:, t * 512:(t + 1) * 512],
                    G[g][:, off:off + 128],
                    G[0][:, t * 512:(t + 1) * 512],
                    start=True,
                    stop=True,
                )
            I = ipool.tile([128, TW], i16, tag="I", name="I")
            nc.vector.tensor_scalar(
                I[:], Pt[:], SCH_A16, SCH_B16, op0=ALU.mult, op1=ALU.add
            )
            first = off_cnt[0] % RB == 0
            last = off_cnt[0] % RB == RB - 1
            for t in range(2):
                mi = nc.tensor.matmul(
                    OFFP[0:1, t * 512:(t + 1) * 512],
                    onesb[:, 0:1],
                    I[:, t * 512:(t + 1) * 512].bitcast(bf16),
                    start=first,
                    stop=last,
                    skip_group_check=True,
                )
                ones_mms.append(mi)
            off_cnt[0] += 1
            if last:
                # drain this OFF_J group's strip to DRAM right away
                rnd = off_cnt[0] // RB - 1
                OFFB = stat.tile([128, TW], f32, tag=f"OFFB{rnd}",
                                 name=f"OFFB{rnd}")
                cp_i = nc.vector.tensor_copy(OFFB[0:1, :], OFFP[0:1, :])
                for m in ones_mms:
                    _dep(cp_i, m, "OFFB reads the accumulated ones-matmuls")
                ones_mms.clear()
                fs_i = nc.gpsimd.dma_start(
                    offd[rnd:rnd + 1, :], OFFB[0:1, :]
                )
                off_stores.append((rnd, fs_i))

        # phases sized so G chunks arrive in time but stationaries are reused
        for phase in ((0, 1), (2, 3, 4, 5)):
            for b in range(RB):
                for j in phase:
                    if j in OFF_J:
                        trans_tile(j * 8 + b)
                    else:
                        row_tile(b, j)
        for b in range(RB):
            row_tile(b, 6)
            trans_tile(7 * 8 + b)

        # ---- fold the transposed-path sums back into row layout ----
        OFF2 = stat.tile([128, 3 * RB], f32)
        for rnd in range(3):
            fl_i = nc.gpsimd.dma_start(
                OFF2[:, rnd * RB:(rnd + 1) * RB].rearrange(
                    "p (q b) -> p q b", q=1),
                offd[rnd:rnd + 1, :].rearrange("q (b p) -> p q b", p=128),
            )
            for srnd, s in off_stores:
                if srnd == rnd:
                    _dep(fl_i, s, "flatten load reads offsums dram")
        OFFR = stat.tile([128, RB], f32)
        nc.vector.reduce_sum(
            OFFR[:], OFF2[:].rearrange("p (q b) -> p b q", q=3), axis=X
        )

        # ---- epilogue: loss = ln(RSC / posE), log via bit-trick + Newton ----
        RS = stat.tile([128, RB], f32)
        red_i = nc.vector.reduce_sum(
            RS[:], SUMS[:].rearrange("p (b j) -> p b j", j=NJ), axis=X
        )
        for a in accum_insts:
            _dep(red_i, a, "RS reads accum sums")
        RSB = stat.tile([128, RB], f32)
        rsb_i = nc.vector.tensor_add(RSB[:], RS[:], OFFR[:])
        _dep(rsb_i, fl_i, "RSB reads flattened offload sums")
        SP = stat.tile([128, RB], f32)
        sp_i = nc.vector.tensor_add(SP[:], SELFE[:], POSE[:])
        for e in extract_insts:
            _dep(sp_i, e, "SP reads diag extracts")
        RSC = stat.tile([128, RB], f32)
        nc.vector.tensor_sub(RSC[:], RSB[:], SP[:])
        # ratio = RSC / posE  (fast reciprocal, ~51 ULP)
        RP = stat.tile([128, RB], f32)
        rp_i = nc.vector.reciprocal_approx_fast(RP[:], POSE[:])
        for e in extract_insts:
            _dep(rp_i, e, "recip reads POSE")
        RT = stat.tile([128, RB], f32)
        nc.vector.tensor_mul(RT[:], RSC[:], RP[:])
        # y0 = bits(ratio)*K - C0 ~ ln(ratio)
        Y0 = stat.tile([128, RB], f32)
        nc.vector.tensor_scalar(
            Y0[:], RT[:].bitcast(i32), LOG_K, -LOG_C0, op0=ALU.mult, op1=ALU.add
        )
        # Newton: loss = y0 - 1 + ratio * exp(-y0)
        EY = stat.tile([128, RB], f32)
        nc.scalar.activation(EY[:], Y0[:], AF.Exp, scale=-1.0)
        T1 = stat.tile([128, RB], f32)
        nc.vector.tensor_mul(T1[:], RT[:], EY[:])
        LOSS = stat.tile([128, RB], f32)
        nc.vector.scalar_tensor_tensor(
            LOSS[:], Y0[:], -1.0, T1[:], ALU.add, ALU.add
        )
        nc.gpsimd.dma_start(out_d[:, :], LOSS[:])

    nc.compile()
    _CACHE["nc"] = nc
    return nc


def kernel(z1: np.ndarray, z2: np.ndarray) -> np.ndarray:
    global LAST_RESULTS
    import ml_dtypes
    from concourse.bass_utils import run_bass_kernel_spmd

    z1 = np.ascontiguousarray(np.asarray(z1, dtype=np.float32))
    z2 = np.ascontiguousarray(np.asarray(z2, dtype=np.float32))
    feats = np.concatenate([z1, z2], axis=0)
    feats_bf = feats.astype(ml_dtypes.bfloat16)
    eyeb = np.eye(128, dtype=ml_dtypes.bfloat16)

    in_maps = []
    for c in range(NCORES):
        fb = np.ascontiguousarray(np.roll(feats_bf, -c * RPC, axis=0))
        in_maps.append({"featsb": fb, "eyeb": eyeb})

    nc = _build()
    res = run_bass_kernel_spmd(nc, in_maps, core_ids=list(range(NCORES)))
    LAST_RESULTS = res

    total = 0.0
    for r in res.results:
        total += float(r["loss_rows"].astype(np.float64).sum())
    return np.float32(total / N2)


# revision 16
# speedup vs baseline: 1.0325x; 1.0325x over previous
"""Trainium2 Bass kernel for the DCL contrastive loss (nn_DCL_11776800325979).

Reference computation:
    feats = concat([z1, z2])                       # [8192, 128]
    cos = (feats @ feats.T) / max(|f_i||f_j|, eps) # [8192, 8192]
    cos[i,i] = -9e15 ; s = cos / 0.1
    pos_i = s[i, (i+4096) % 8192]
    neg = s with the pos column also masked
    loss = mean(-pos_i + logsumexp(neg, axis=-1))

Strategy (8 NeuronCores, data-parallel over rows):
  Each core receives feats rolled by -c*1024 rows (bf16) and computes the
  loss rows for *local* rows 0..1023 against all 8192 columns.  Rolling
  both index spaces by the same amount preserves the self offset (i==j)
  and the positive-pair offset ((i+4096) mod 8192), so one NEFF runs SPMD
  on all 8 cores.

  Preamble (per 2048-row chunk, pipelined): row-major bf16 half-loads on
  both HWDGE rings (contiguous 4KB/partition descriptors), bf16
  square+reduce -> |f|^2, bf16 magic-seed Newton rsqrt (1 iter) ->
  1/|f|, one broadcast-multiply normalizes the rows (chunk 0 on DVE,
  rest on Pool), store to a DRAM scratch + xbar DMA-transpose reload
  (both on the sync HWDGE ring) -> normalized G^T chunks in SBUF.

  Main loop: ACT covers most column tiles with exp(10*cos) [128,1024]
  activations + fused row-sum accumulators.  OFF_J column tiles are
  computed *transposed* (cols as partitions): DVE turns the PSUM logits
  into bf16 Schraudolph exp bits (one tensor_scalar), and the PE
  row-sums them via ones-matmuls accumulated into a [1,1024] PSUM strip
  that is folded back into the row sums at the end.  Self/pos diagonal
  terms are extracted from the ACT bf16 exp output (eyeb multiply on
  Pool, row-reduce on DVE) and subtracted.

  epilogue: loss rows = ln(RSC/posE) via a DVE log bit-trick plus one
  exp-based Newton refinement, so the Exp activation table loaded in the
  preamble is the only table load.  Host averages.
"""

import math

import numpy as np

D = 128          # feature dim (= contraction dim = partitions)
N2 = 8192        # 2N rows
NCORES = 8
RPC = N2 // NCORES          # rows per core = 1024
RB = RPC // 128             # row blocks per core = 8
CG = 4                      # chunks of 2048 (both row-chunks and G-chunks)
CGW = N2 // CG              # chunk width = 2048
TW = 1024                   # logit tile width
NJ = N2 // TW               # 8 column tiles per row block
INV_TEMP = 10.0
MAGIC16 = 0x5F37            # fast inverse sqrt seed (bf16 bits)
LN2 = math.log(2.0)

# bf16 Schraudolph exp: i16 bits = round(cos * SCH_A16 + SCH_B16);
# bitcast bf16 ~ exp(10*cos)
SCH_A16 = (2.0 ** 7) * INV_TEMP / LN2
SCH_B16 = (127.0 - 0.0564) * (2.0 ** 7)
# f32 log bit-trick: ln(x) ~ bits(x) * LOG_K - LOG_C0   (then one Newton)
LOG_K = LN2 / (2.0 ** 23)
LOG_C0 = (127.0 - 0.0430) * LN2

# column tiles computed transposed with PE row-sum accumulation.
# j=0 (self diag) and j=4 (pos diag) must stay on ACT.
OFF_J = (3, 7)

_CACHE = {}
LAST_RESULTS = None


def _build():
    if "nc" in _CACHE:
        return _CACHE["nc"]

    from contextlib import ExitStack

    import concourse.bass as bass  # noqa: F401
    import concourse.mybir as mybir
    import concourse.tile as tile
    from concourse import bacc

    f32 = mybir.dt.float32
    i16 = mybir.dt.int16
    i32 = mybir.dt.int32
    bf16 = mybir.dt.bfloat16
    AF = mybir.ActivationFunctionType
    ALU = mybir.AluOpType
    X = mybir.AxisListType.X

    nc = bacc.Bacc(
        "TRN2",
        target_bir_lowering=False,
        debug=False,
        enable_asserts=False,
        num_devices=NCORES,
    )

    featsb = nc.dram_tensor("featsb", [N2, D], bf16, kind="ExternalInput").ap()
    eyeb_d = nc.dram_tensor("eyeb", [128, 128], bf16, kind="ExternalInput").ap()
    scratch = nc.dram_tensor("scratch", [N2, D], bf16, kind="Internal").ap()
    offd = nc.dram_tensor("offsums", [2, RPC], f32, kind="Internal").ap()
    out_d = nc.dram_tensor("loss_rows", [128, RB], f32, kind="ExternalOutput").ap()

    n_off = len(OFF_J) * RB          # transposed column-block tiles

    with tile.TileContext(nc) as tc, ExitStack() as ctx:
        consts = ctx.enter_context(tc.tile_pool(name="consts", bufs=1))
        gpool = ctx.enter_context(tc.tile_pool(name="G", bufs=1))
        fpool = ctx.enter_context(tc.tile_pool(name="F", bufs=4))
        fnpool = ctx.enter_context(tc.tile_pool(name="FN", bufs=4))
        sqpool = ctx.enter_context(tc.tile_pool(name="SQ", bufs=2))
        scrp = ctx.enter_context(tc.tile_pool(name="scr", bufs=2))
        stat = ctx.enter_context(tc.tile_pool(name="stat", bufs=1))
        epool = ctx.enter_context(tc.tile_pool(name="E", bufs=16))
        ipool = ctx.enter_context(tc.tile_pool(name="I", bufs=4))
        xpool = ctx.enter_context(tc.tile_pool(name="XT", bufs=3))
        ppool = ctx.enter_context(tc.tile_pool(name="P", bufs=3, space="PSUM"))
        opool = ctx.enter_context(tc.tile_pool(name="OP", bufs=1, space="PSUM"))

        def _dep(after, before, reason):
            a = getattr(after, "ins", after)
            b = getattr(before, "ins", before)
            tile.add_dep_helper(a, b, reason=reason)

        eyeb = consts.tile([128, 128], bf16)
        onesb = consts.tile([128, 1], bf16)
        nc.vector.memset(onesb[:], 1.0)
        magicT = consts.tile([128, 16], i16)
        nc.vector.memset(magicT[:], MAGIC16)
        c15 = consts.tile([128, 16], bf16)
        nc.vector.memset(c15[:], 1.5)
        dummy_in = consts.tile([128, 1], f32)
        nc.vector.memset(dummy_in[:], 1.0)
        dummy_out = consts.tile([128, 1], f32)

        SS = stat.tile([128, 64], bf16)      # per-row |f|^2 (col t = row tile)
        RNb = stat.tile([128, 64], bf16)     # 1/|f| (bf16)
        SUMS = stat.tile([128, RB * NJ], f32)   # row sumexp per (b, j), ACT js
        nc.vector.memset(SUMS[:], 0.0)
        SELFE = stat.tile([128, RB], f32)    # exp(self) per row block
        POSE = stat.tile([128, RB], f32)     # exp(pos) per row block

        G = [
            gpool.tile([128, CGW], bf16, tag=f"G{g}", name=f"G{g}")
            for g in range(CG)
        ]

        # ---- preamble: half-chunk loads on the two HWDGE rings ----
        # partition p of chunk g holds rows g*2048 + 16p + t (contiguous 4KB)
        Fgs = []
        with tc.high_priority():
            for g in range(CG):
                Fg = fpool.tile([128, CGW], bf16, tag=f"F{g}", name=f"Fg{g}")
                Fgs.append(Fg)
            for g in range(CG):
                for h, eng in ((0, nc.scalar), (1, nc.sync)):
                    # half h = partitions 64h..64h+63 = rows g*2048+1024h..
                    eng.dma_start(
                        Fgs[g][64 * h:64 * (h + 1), :].rearrange(
                            "p (t d) -> p t d", d=128),
                        featsb[g * CGW + h * TW:g * CGW + (h + 1) * TW, :]
                        .rearrange("(p t) d -> p t d", t=16),
                    )
        nc.gpsimd.dma_start(eyeb[:], eyeb_d[:, :])
        # trigger the activation table load during the preamble; Square
        # then Exp narrows the choice to a set that serves both (6 or 22)
        nc.scalar.activation(dummy_out[:], dummy_in[:], AF.Square)
        nc.scalar.activation(dummy_out[:], dummy_in[:], AF.Exp)

        def rsqrt_chunk(g):
            """RNb[:, g*16:(g+1)*16] = 1/sqrt(SS[..]), bf16, 1 Newton iter."""
            lo, hi = g * 16, (g + 1) * 16
            x = SS[:, lo:hi]
            y = scrp.tile([128, 16], bf16, tag="nw_y", name="nw_y")
            t = scrp.tile([128, 16], bf16, tag="nw_t", name="nw_t")
            nc.vector.tensor_scalar(
                y[:].bitcast(i16), x.bitcast(i16), 1, None,
                op0=ALU.logical_shift_right,
            )
            nc.vector.tensor_sub(y[:].bitcast(i16), magicT[:], y[:].bitcast(i16))
            nc.vector.tensor_mul(t[:], y[:], y[:])
            nc.vector.tensor_mul(t[:], t[:], x)
            nc.vector.scalar_tensor_tensor(
                t[:], t[:], -0.5, c15[:], ALU.mult, ALU.add
            )
            nc.vector.tensor_mul(RNb[:, lo:hi], y[:], t[:])

        # ---- per-chunk: norms -> normalize -> store -> transposed load ----
        for g in range(CG):
            from contextlib import nullcontext
            prio = tc.high_priority() if g == 0 else nullcontext()
            with prio:
                Fg = Fgs[g]
                SQ = sqpool.tile([128, CGW], bf16, tag="SQ", name="SQ")
                nc.scalar.activation(SQ[:], Fg[:], AF.Square)
                with nc.allow_low_precision(
                    reason="|f|^2 in bf16 is plenty for a 1/|f| scale"
                ):
                    nc.vector.reduce_sum(
                        SS[:, g * 16:(g + 1) * 16],
                        SQ[:].rearrange("p (t d) -> p t d", d=128), axis=X,
                    )
                rsqrt_chunk(g)
                Fn = fnpool.tile([128, CGW], bf16, tag=f"FN{g}", name=f"Fn{g}")
                mul_eng = nc.vector
                mul_eng.tensor_mul(
                    Fn[:].rearrange("p (t d) -> p t d", d=128),
                    Fg[:].rearrange("p (t d) -> p t d", d=128),
                    RNb[:, g * 16:(g + 1) * 16, None].to_broadcast((128, 16, 128)),
                )
                if g == 0:
                    # halve the DRAM roundtrip: rows 0..1023 (partitions
                    # 0..63) first so phase A's first tiles start sooner
                    for h in range(2):
                        st_i = nc.sync.dma_start(
                            scratch[h * TW:(h + 1) * TW, :].rearrange(
                                "(p t) d -> p t d", t=16),
                            Fn[64 * h:64 * (h + 1), :].rearrange(
                                "p (t d) -> p t d", d=128),
                        )
                        tl_i = nc.sync.dma_start(
                            G[0][:, h * TW:(h + 1) * TW],
                            scratch[h * TW:(h + 1) * TW, :],
                            transpose=True,
                        )
                        _dep(tl_i, st_i, "transpose load reads scratch half")
                else:
                    st_i = nc.sync.dma_start(
                        scratch[g * CGW:(g + 1) * CGW, :].rearrange(
                            "(p t) d -> p t d", t=16),
                        Fn[:].rearrange("p (t d) -> p t d", d=128),
                    )
                    tl_i = nc.sync.dma_start(
                        G[g][:], scratch[g * CGW:(g + 1) * CGW, :],
                        transpose=True,
                    )
                    _dep(tl_i, st_i, "transpose load reads scratch chunk")

        # ---- main loop ----
        accum_insts = []
        extract_insts = []
        OFFP = opool.tile([128, TW], f32, tag="OFFP", name="OFFP")
        off_cnt = [0]
        ones_mms = []
        off_stores = []

        def row_tile(b, j):
            P = ppool.tile([128, TW], f32, tag="P", name="P")
            for t in range(2):
                off = (j % 2) * TW + t * 512
                nc.tensor.matmul(
                    P[:, t * 512:(t + 1) * 512],
                    G[0][:, b * 128:(b + 1) * 128],
                    G[j // 2][:, off:off + 512],
                    start=True,
                    stop=True,
                )
            k = b * NJ + j
            E = epool.tile([128, TW], bf16, tag="E", name="E")
            act_i = nc.scalar.activation(
                E[:], P[:], AF.Exp, scale=INV_TEMP,
                accum_out=SUMS[:, k:k + 1],
            )
            accum_insts.append(act_i)
            if j == 0 or j == 4:
                # diagonal of this block holds exp(self)/exp(pos)
                dst = SELFE if j == 0 else POSE
                xr = xpool.tile([128, 128], bf16, tag="xs", name="xs")
                nc.gpsimd.tensor_mul(
                    xr[:], E[:, b * 128:(b + 1) * 128], eyeb[:]
                )
                e2 = nc.vector.reduce_sum(dst[:, b:b + 1], xr[:], axis=X)
                extract_insts.append(e2)

        def trans_tile(cb):
            """Column block cb (128 cols) vs all 1024 core rows, transposed.
            exp via bf16 Schraudolph bits on DVE; row sums via PE
            ones-matmuls accumulated into OFFP[0, :]."""
            g, off = cb // 16, (cb % 16) * 128
            Pt = ppool.tile([128, TW], f32, tag="P", name="Pt")
            for t in range(2):
                nc.tensor.matmul(
                    Pt[:, t * 512:(t + 1) * 512],
                    G[g][:, off:off + 128],
                    G[0][:, t * 512:(t + 1) * 512],
                    start=True,
                    stop=True,
                )
            I = ipool.tile([128, TW], i16, tag="I", name="I")
            nc.vector.tensor_scalar(
                I[:], Pt[:], SCH_A16, SCH_B16, op0=ALU.mult, op1=ALU.add
            )
            first = off_cnt[0] % RB == 0
            last = off_cnt[0] % RB == RB - 1
            for t in range(2):
                mi = nc.tensor.matmul(
                    OFFP[0:1, t * 512:(t + 1) * 512],
                    onesb[:, 0:1],
                    I[:, t * 512:(t + 1) * 512].bitcast(bf16),
                    start=first,
                    stop=last,
                    skip_group_check=True,
                )
                ones_mms.append(mi)
            off_cnt[0] += 1
            if last:
                # drain this OFF_J group's strip to DRAM right away
                rnd = off_cnt[0] // RB - 1
                OFFB = stat.tile([128, TW], f32, tag=f"OFFB{rnd}",
                                 name=f"OFFB{rnd}")
                cp_i = nc.vector.tensor_copy(OFFB[0:1, :], OFFP[0:1, :])
                for m in ones_mms:
                    _dep(cp_i, m, "OFFB reads the accumulated ones-matmuls")
                ones_mms.clear()
                fs_i = nc.gpsimd.dma_start(
                    offd[rnd:rnd + 1, :], OFFB[0:1, :]
                )
                off_stores.append((rnd, fs_i))

        # phases sized so G chunks arrive in time but stationaries are reused
        for phase in ((0, 1), (2, 3, 4, 5)):
            for b in range(RB):
                for j in phase:
                    if j in OFF_J:
                        trans_tile(j * 8 + b)
                    else:
                        row_tile(b, j)
        for b in range(RB):
            row_tile(b, 6)
            trans_tile(7 * 8 + b)

        # ---- fold the transposed-path sums back into row layout ----
        OFF2 = stat.tile([128, 2 * RB], f32)
        for rnd in range(2):
            fl_i = nc.gpsimd.dma_start(
                OFF2[:, rnd * RB:(rnd + 1) * RB].rearrange(
                    "p (q b) -> p q b", q=1),
                offd[rnd:rnd + 1, :].rearrange("q (b p) -> p q b", p=128),
            )
            for srnd, s in off_stores:
                if srnd == rnd:
                    _dep(fl_i, s, "flatten load reads offsums dram")
        OFFR = stat.tile([128, RB], f32)
        nc.vector.reduce_sum(
            OFFR[:], OFF2[:].rearrange("p (q b) -> p b q", q=2), axis=X
        )

        # ---- epilogue: loss = ln(RSC / posE), log via bit-trick + Newton ----
        RS = stat.tile([128, RB], f32)
        red_i = nc.vector.reduce_sum(
            RS[:], SUMS[:].rearrange("p (b j) -> p b j", j=NJ), axis=X
        )
        for a in accum_insts:
            _dep(red_i, a, "RS reads accum sums")
        RSB = stat.tile([128, RB], f32)
        rsb_i = nc.vector.tensor_add(RSB[:], RS[:], OFFR[:])
        _dep(rsb_i, fl_i, "RSB reads flattened offload sums")
        SP = stat.tile([128, RB], f32)
        sp_i = nc.vector.tensor_add(SP[:], SELFE[:], POSE[:])
        for e in extract_insts:
            _dep(sp_i, e, "SP reads diag extracts")
        RSC = stat.tile([128, RB], f32)
        nc.vector.tensor_sub(RSC[:], RSB[:], SP[:])
        # ratio = RSC / posE  (fast reciprocal, ~51 ULP)
        RP = stat.tile([128, RB], f32)
        rp_i = nc.vector.reciprocal_approx_fast(RP[:], POSE[:])
        for e in extract_insts:
            _dep(rp_i, e, "recip reads POSE")
        RT = stat.tile([128, RB], f32)
        nc.vector.tensor_mul(RT[:], RSC[:], RP[:])
        # y0 = bits(ratio)*K - C0 ~ ln(ratio)
        Y0 = stat.tile([128, RB], f32)
        nc.vector.tensor_scalar(
            Y0[:], RT[:].bitcast(i32), LOG_K, -LOG_C0, op0=ALU.mult, op1=ALU.add
        )
        # Newton: loss = y0 - 1 + ratio * exp(-y0)
        EY = stat.tile([128, RB], f32)
        nc.scalar.activation(EY[:], Y0[:], AF.Exp, scale=-1.0)
        T1 = stat.tile([128, RB], f32)
        nc.vector.tensor_mul(T1[:], RT[:], EY[:])
        LOSS = stat.tile([128, RB], f32)
        nc.vector.scalar_tensor_tensor(
            LOSS[:], Y0[:], -1.0, T1[:], ALU.add, ALU.add
        )
        nc.gpsimd.dma_start(out_d[:, :], LOSS[:])

    nc.compile()
    _CACHE["nc"] = nc
    return nc


def kernel(z1: np.ndarray, z2: np.ndarray) -> np.ndarray:
    global LAST_RESULTS
    import ml_dtypes
    from concourse.bass_utils import run_bass_kernel_spmd

    z1 = np.ascontiguousarray(np.asarray(z1, dtype=np.float32))
    z2 = np.ascontiguousarray(np.asarray(z2, dtype=np.float32))
    feats = np.concatenate([z1, z2], axis=0)
    feats_bf = feats.astype(ml_dtypes.bfloat16)
    eyeb = np.eye(128, dtype=ml_dtypes.bfloat16)

    in_maps = []
    for c in range(NCORES):
        fb = np.ascontiguousarray(np.roll(feats_bf, -c * RPC, axis=0))
        in_maps.append({"featsb": fb, "eyeb": eyeb})

    nc = _build()
    res = run_bass_kernel_spmd(nc, in_maps, core_ids=list(range(NCORES)))
    LAST_RESULTS = res

    total = 0.0
    for r in res.results:
        total += float(r["loss_rows"].astype(np.float64).sum())
    return np.float32(total / N2)


# revision 19
# speedup vs baseline: 1.0903x; 1.0559x over previous
"""Trainium2 Bass kernel for the DCL contrastive loss (nn_DCL_11776800325979).

Reference computation:
    feats = concat([z1, z2])                       # [8192, 128]
    cos = (feats @ feats.T) / max(|f_i||f_j|, eps) # [8192, 8192]
    cos[i,i] = -9e15 ; s = cos / 0.1
    pos_i = s[i, (i+4096) % 8192]
    neg = s with the pos column also masked
    loss = mean(-pos_i + logsumexp(neg, axis=-1))

Strategy (8 NeuronCores, data-parallel over rows):
  Each core receives feats rolled by -c*1024 rows (bf16) and computes the
  loss rows for *local* rows 0..1023 against all 8192 columns.  Rolling
  both index spaces by the same amount preserves the self offset (i==j)
  and the positive-pair offset ((i+4096) mod 8192), so one NEFF runs SPMD
  on all 8 cores.

  Preamble (per 2048-row chunk, pipelined): row-major bf16 half-loads on
  both HWDGE rings (contiguous 4KB/partition descriptors), bf16
  square+reduce -> |f|^2, bf16 magic-seed Newton rsqrt (1 iter) ->
  1/|f|, one broadcast-multiply normalizes the rows (chunk 0 on DVE,
  rest on Pool), store to a DRAM scratch + xbar DMA-transpose reload
  (both on the sync HWDGE ring) -> normalized G^T chunks in SBUF.

  Main loop: ACT covers most column tiles with exp(10*cos) [128,1024]
  activations + fused row-sum accumulators.  OFF_J column tiles are
  computed *transposed* (cols as partitions): DVE turns the PSUM logits
  into bf16 Schraudolph exp bits (one tensor_scalar), and the PE
  row-sums them via ones-matmuls accumulated into a [1,1024] PSUM strip
  that is folded back into the row sums at the end.  Self/pos diagonal
  terms are extracted from the ACT bf16 exp output (eyeb multiply on
  Pool, row-reduce on DVE) and subtracted.

  epilogue: loss rows = ln(RSC/posE) via a DVE log bit-trick plus one
  exp-based Newton refinement, so the Exp activation table loaded in the
  preamble is the only table load.  Host averages.
"""

import math

import numpy as np

D = 128          # feature dim (= contraction dim = partitions)
N2 = 8192        # 2N rows
NCORES = 8
RPC = N2 // NCORES          # rows per core = 1024
RB = RPC // 128             # row blocks per core = 8
CG = 4                      # chunks of 2048 (both row-chunks and G-chunks)
CGW = N2 // CG              # chunk width = 2048
TW = 1024                   # logit tile width
NJ = N2 // TW               # 8 column tiles per row block
INV_TEMP = 10.0
MAGIC16 = 0x5F37            # fast inverse sqrt seed (bf16 bits)
LN2 = math.log(2.0)

# bf16 Schraudolph exp: i16 bits = round(cos * SCH_A16 + SCH_B16);
# bitcast bf16 ~ exp(10*cos)
SCH_A16 = (2.0 ** 7) * INV_TEMP / LN2
SCH_B16 = (127.0 - 0.0564) * (2.0 ** 7)
# f32 log bit-trick: ln(x) ~ bits(x) * LOG_K - LOG_C0   (then one Newton)
LOG_K = LN2 / (2.0 ** 23)
LOG_C0 = (127.0 - 0.0430) * LN2

# column tiles computed transposed with PE row-sum accumulation.
# j=0 (self diag) and j=4 (pos diag) must stay on ACT.
OFF_J = (3, 5, 7)

_CACHE = {}
LAST_RESULTS = None


def _build():
    if "nc" in _CACHE:
        return _CACHE["nc"]

    from contextlib import ExitStack

    import concourse.bass as bass  # noqa: F401
    import concourse.mybir as mybir
    import concourse.tile as tile
    from concourse import bacc

    f32 = mybir.dt.float32
    i16 = mybir.dt.int16
    i32 = mybir.dt.int32
    bf16 = mybir.dt.bfloat16
    AF = mybir.ActivationFunctionType
    ALU = mybir.AluOpType
    X = mybir.AxisListType.X

    nc = bacc.Bacc(
        "TRN2",
        target_bir_lowering=False,
        debug=False,
        enable_asserts=False,
        num_devices=NCORES,
    )

    featsb = nc.dram_tensor("featsb", [N2, D], bf16, kind="ExternalInput").ap()
    eyeb_d = nc.dram_tensor("eyeb", [128, 128], bf16, kind="ExternalInput").ap()
    scratch = nc.dram_tensor("scratch", [N2, D], bf16, kind="Internal").ap()
    offd = nc.dram_tensor("offsums", [3, RPC], f32, kind="Internal").ap()
    out_d = nc.dram_tensor("loss_rows", [128, RB], f32, kind="ExternalOutput").ap()

    n_off = len(OFF_J) * RB          # transposed column-block tiles

    with tile.TileContext(nc) as tc, ExitStack() as ctx:
        consts = ctx.enter_context(tc.tile_pool(name="consts", bufs=1))
        gpool = ctx.enter_context(tc.tile_pool(name="G", bufs=1))
        fpool = ctx.enter_context(tc.tile_pool(name="F", bufs=4))
        fnpool = ctx.enter_context(tc.tile_pool(name="FN", bufs=4))
        sqpool = ctx.enter_context(tc.tile_pool(name="SQ", bufs=2))
        scrp = ctx.enter_context(tc.tile_pool(name="scr", bufs=2))
        stat = ctx.enter_context(tc.tile_pool(name="stat", bufs=1))
        epool = ctx.enter_context(tc.tile_pool(name="E", bufs=16))
        ipool = ctx.enter_context(tc.tile_pool(name="I", bufs=4))
        xpool = ctx.enter_context(tc.tile_pool(name="XT", bufs=3))
        ppool = ctx.enter_context(tc.tile_pool(name="P", bufs=3, space="PSUM"))
        opool = ctx.enter_context(tc.tile_pool(name="OP", bufs=1, space="PSUM"))

        def _dep(after, before, reason):
            a = getattr(after, "ins", after)
            b = getattr(before, "ins", before)
            tile.add_dep_helper(a, b, reason=reason)

        eyeb = consts.tile([128, 128], bf16)
        onesb = consts.tile([128, 1], bf16)
        nc.vector.memset(onesb[:], 1.0)
        magicT = consts.tile([128, 16], i16)
        nc.vector.memset(magicT[:], MAGIC16)
        c15 = consts.tile([128, 16], bf16)
        nc.vector.memset(c15[:], 1.5)
        dummy_in = consts.tile([128, 1], f32)
        nc.vector.memset(dummy_in[:], 1.0)
        dummy_out = consts.tile([128, 1], f32)

        SS = stat.tile([128, 64], bf16)      # per-row |f|^2 (col t = row tile)
        RNb = stat.tile([128, 64], bf16)     # 1/|f| (bf16)
        SUMS = stat.tile([128, RB * NJ], f32)   # row sumexp per (b, j), ACT js
        nc.vector.memset(SUMS[:], 0.0)
        SELFE = stat.tile([128, RB], f32)    # exp(self) per row block
        POSE = stat.tile([128, RB], f32)     # exp(pos) per row block

        G = [
            gpool.tile([128, CGW], bf16, tag=f"G{g}", name=f"G{g}")
            for g in range(CG)
        ]

        # ---- preamble: half-chunk loads on the two HWDGE rings ----
        # partition p of chunk g holds rows g*2048 + 16p + t (contiguous 4KB)
        Fgs = []
        with tc.high_priority():
            for g in range(CG):
                Fg = fpool.tile([128, CGW], bf16, tag=f"F{g}", name=f"Fg{g}")
                Fgs.append(Fg)
            for g in range(CG):
                for h, eng in ((0, nc.scalar), (1, nc.sync)):
                    # half h = partitions 64h..64h+63 = rows g*2048+1024h..
                    eng.dma_start(
                        Fgs[g][64 * h:64 * (h + 1), :].rearrange(
                            "p (t d) -> p t d", d=128),
                        featsb[g * CGW + h * TW:g * CGW + (h + 1) * TW, :]
                        .rearrange("(p t) d -> p t d", t=16),
                    )
        nc.gpsimd.dma_start(eyeb[:], eyeb_d[:, :])
        # trigger the activation table load during the preamble; Square
        # then Exp narrows the choice to a set that serves both (6 or 22)
        nc.scalar.activation(dummy_out[:], dummy_in[:], AF.Square)
        nc.scalar.activation(dummy_out[:], dummy_in[:], AF.Exp)

        def rsqrt_chunk(g):
            """RNb[:, g*16:(g+1)*16] = 1/sqrt(SS[..]), bf16, 1 Newton iter."""
            lo, hi = g * 16, (g + 1) * 16
            x = SS[:, lo:hi]
            y = scrp.tile([128, 16], bf16, tag="nw_y", name="nw_y")
            t = scrp.tile([128, 16], bf16, tag="nw_t", name="nw_t")
            nc.vector.tensor_scalar(
                y[:].bitcast(i16), x.bitcast(i16), 1, None,
                op0=ALU.logical_shift_right,
            )
            nc.vector.tensor_sub(y[:].bitcast(i16), magicT[:], y[:].bitcast(i16))
            nc.vector.tensor_mul(t[:], y[:], y[:])
            nc.vector.tensor_mul(t[:], t[:], x)
            nc.vector.scalar_tensor_tensor(
                t[:], t[:], -0.5, c15[:], ALU.mult, ALU.add
            )
            nc.vector.tensor_mul(RNb[:, lo:hi], y[:], t[:])

        # ---- per-chunk: norms -> normalize -> store -> transposed load ----
        for g in range(CG):
            from contextlib import nullcontext
            prio = tc.high_priority() if g == 0 else nullcontext()
            with prio:
                Fg = Fgs[g]
                SQ = sqpool.tile([128, CGW], bf16, tag="SQ", name="SQ")
                nc.scalar.activation(SQ[:], Fg[:], AF.Square)
                with nc.allow_low_precision(
                    reason="|f|^2 in bf16 is plenty for a 1/|f| scale"
                ):
                    nc.vector.reduce_sum(
                        SS[:, g * 16:(g + 1) * 16],
                        SQ[:].rearrange("p (t d) -> p t d", d=128), axis=X,
                    )
                rsqrt_chunk(g)
                Fn = fnpool.tile([128, CGW], bf16, tag=f"FN{g}", name=f"Fn{g}")
                mul_eng = nc.vector
                mul_eng.tensor_mul(
                    Fn[:].rearrange("p (t d) -> p t d", d=128),
                    Fg[:].rearrange("p (t d) -> p t d", d=128),
                    RNb[:, g * 16:(g + 1) * 16, None].to_broadcast((128, 16, 128)),
                )
                if g == 0:
                    # halve the DRAM roundtrip: rows 0..1023 (partitions
                    # 0..63) first so phase A's first tiles start sooner
                    for h in range(2):
                        st_i = nc.sync.dma_start(
                            scratch[h * TW:(h + 1) * TW, :].rearrange(
                                "(p t) d -> p t d", t=16),
                            Fn[64 * h:64 * (h + 1), :].rearrange(
                                "p (t d) -> p t d", d=128),
                        )
                        tl_i = nc.sync.dma_start(
                            G[0][:, h * TW:(h + 1) * TW],
                            scratch[h * TW:(h + 1) * TW, :],
                            transpose=True,
                        )
                        _dep(tl_i, st_i, "transpose load reads scratch half")
                else:
                    st_i = nc.sync.dma_start(
                        scratch[g * CGW:(g + 1) * CGW, :].rearrange(
                            "(p t) d -> p t d", t=16),
                        Fn[:].rearrange("p (t d) -> p t d", d=128),
                    )
                    tl_i = nc.sync.dma_start(
                        G[g][:], scratch[g * CGW:(g + 1) * CGW, :],
                        transpose=True,
                    )
                    _dep(tl_i, st_i, "transpose load reads scratch chunk")

        # ---- main loop ----
        accum_insts = []
        extract_insts = []
        OFFP = opool.tile([128, TW], f32, tag="OFFP", name="OFFP")
        off_cnt = [0]
        ones_mms = []
        off_stores = []

        def row_tile(b, j):
            P = ppool.tile([128, TW], f32, tag="P", name="P")
            for t in range(2):
                off = (j % 2) * TW + t * 512
                nc.tensor.matmul(
                    P[:, t * 512:(t + 1) * 512],
                    G[0][:, b * 128:(b + 1) * 128],
                    G[j // 2][:, off:off + 512],
                    start=True,
                    stop=True,
                )
            k = b * NJ + j
            E = epool.tile([128, TW], bf16, tag="E", name="E")
            act_i = nc.scalar.activation(
                E[:], P[:], AF.Exp, scale=INV_TEMP,
                accum_out=SUMS[:, k:k + 1],
            )
            accum_insts.append(act_i)
            if j == 0 or j == 4:
                # diagonal of this block holds exp(self)/exp(pos)
                dst = SELFE if j == 0 else POSE
                xr = xpool.tile([128, 128], bf16, tag="xs", name="xs")
                nc.gpsimd.tensor_mul(
                    xr[:], E[:, b * 128:(b + 1) * 128], eyeb[:]
                )
                e2 = nc.vector.reduce_sum(dst[:, b:b + 1], xr[:], axis=X)
                extract_insts.append(e2)

        def trans_tile(cb):
            """Column block cb (128 cols) vs all 1024 core rows, transposed.
            exp via bf16 Schraudolph bits on DVE; row sums via PE
            ones-matmuls accumulated into OFFP[0, :]."""
            g, off = cb // 16, (cb % 16) * 128
            Pt = ppool.tile([128, TW], f32, tag="P", name="Pt")
            for t in range(2):
                nc.tensor.matmul(
                    Pt[:, t * 512:(t + 1) * 512],
                    G[g][:, off:off + 128],
                    G[0][:, t * 512:(t + 1) * 512],
                    start=True,
                    stop=True,
                )
            I = ipool.tile([128, TW], i16, tag="I", name="I")
            nc.vector.tensor_scalar(
                I[:], Pt[:], SCH_A16, SCH_B16, op0=ALU.mult, op1=ALU.add
            )
            first = off_cnt[0] % RB == 0
            last = off_cnt[0] % RB == RB - 1
            for t in range(2):
                mi = nc.tensor.matmul(
                    OFFP[0:1, t * 512:(t + 1) * 512],
                    onesb[:, 0:1],
                    I[:, t * 512:(t + 1) * 512].bitcast(bf16),
                    start=first,
                    stop=last,
                    skip_group_check=True,
                )
                ones_mms.append(mi)
            off_cnt[0] += 1
            if last:
                # drain this OFF_J group's strip to DRAM right away
                rnd = off_cnt[0] // RB - 1
                OFFB = stat.tile([128, TW], f32, tag=f"OFFB{rnd}",
                                 name=f"OFFB{rnd}")
                cp_i = nc.vector.tensor_copy(OFFB[0:1, :], OFFP[0:1, :])
                for m in ones_mms:
                    _dep(cp_i, m, "OFFB reads the accumulated ones-matmuls")
                ones_mms.clear()
                fs_i = nc.gpsimd.dma_start(
                    offd[rnd:rnd + 1, :], OFFB[0:1, :]
                )
                off_stores.append((rnd, fs_i))

        # phases sized so G chunks arrive in time but stationaries are reused
        for phase in ((0, 1), (2, 3, 4, 5)):
            for b in range(RB):
                for j in phase:
                    if j in OFF_J:
                        trans_tile(j * 8 + b)
                    else:
                        row_tile(b, j)
        for b in range(RB):
            trans_tile(7 * 8 + b)
        for b in range(RB):
            row_tile(b, 6)

        # ---- fold the transposed-path sums back into row layout ----
        OFF2 = stat.tile([128, 3 * RB], f32)
        for rnd in range(3):
            fl_i = nc.gpsimd.dma_start(
                OFF2[:, rnd * RB:(rnd + 1) * RB].rearrange(
                    "p (q b) -> p q b", q=1),
                offd[rnd:rnd + 1, :].rearrange("q (b p) -> p q b", p=128),
            )
            for srnd, s in off_stores:
                if srnd == rnd:
                    _dep(fl_i, s, "flatten load reads offsums dram")
        OFFR = stat.tile([128, RB], f32)
        nc.vector.reduce_sum(
            OFFR[:], OFF2[:].rearrange("p (q b) -> p b q", q=3), axis=X
        )

        # ---- epilogue: loss = ln(RSC / posE), log via bit-trick + Newton ----
        RS = stat.tile([128, RB], f32)
        red_i = nc.vector.reduce_sum(
            RS[:], SUMS[:].rearrange("p (b j) -> p b j", j=NJ), axis=X
        )
        for a in accum_insts:
            _dep(red_i, a, "RS reads accum sums")
        RSB = stat.tile([128, RB], f32)
        rsb_i = nc.vector.tensor_add(RSB[:], RS[:], OFFR[:])
        _dep(rsb_i, fl_i, "RSB reads flattened offload sums")
        SP = stat.tile([128, RB], f32)
        sp_i = nc.vector.tensor_add(SP[:], SELFE[:], POSE[:])
        for e in extract_insts:
            _dep(sp_i, e, "SP reads diag extracts")
        RSC = stat.tile([128, RB], f32)
        nc.vector.tensor_sub(RSC[:], RSB[:], SP[:])
        # ratio = RSC / posE  (fast reciprocal, ~51 ULP)
        RP = stat.tile([128, RB], f32)
        rp_i = nc.vector.reciprocal_approx_fast(RP[:], POSE[:])
        for e in extract_insts:
            _dep(rp_i, e, "recip reads POSE")
        RT = stat.tile([128, RB], f32)
        nc.vector.tensor_mul(RT[:], RSC[:], RP[:])
        # y0 = bits(ratio)*K - C0 ~ ln(ratio)
        Y0 = stat.tile([128, RB], f32)
        nc.vector.tensor_scalar(
            Y0[:], RT[:].bitcast(i32), LOG_K, -LOG_C0, op0=ALU.mult, op1=ALU.add
        )
        # Newton: loss = y0 - 1 + ratio * exp(-y0)
        EY = stat.tile([128, RB], f32)
        nc.scalar.activation(EY[:], Y0[:], AF.Exp, scale=-1.0)
        T1 = stat.tile([128, RB], f32)
        nc.vector.tensor_mul(T1[:], RT[:], EY[:])
        LOSS = stat.tile([128, RB], f32)
        nc.vector.scalar_tensor_tensor(
            LOSS[:], Y0[:], -1.0, T1[:], ALU.add, ALU.add
        )
        nc.gpsimd.dma_start(out_d[:, :], LOSS[:])

    nc.compile()
    _CACHE["nc"] = nc
    return nc


def kernel(z1: np.ndarray, z2: np.ndarray) -> np.ndarray:
    global LAST_RESULTS
    import ml_dtypes
    from concourse.bass_utils import run_bass_kernel_spmd

    z1 = np.ascontiguousarray(np.asarray(z1, dtype=np.float32))
    z2 = np.ascontiguousarray(np.asarray(z2, dtype=np.float32))
    feats = np.concatenate([z1, z2], axis=0)
    feats_bf = feats.astype(ml_dtypes.bfloat16)
    eyeb = np.eye(128, dtype=ml_dtypes.bfloat16)

    in_maps = []
    for c in range(NCORES):
        fb = np.ascontiguousarray(np.roll(feats_bf, -c * RPC, axis=0))
        in_maps.append({"featsb": fb, "eyeb": eyeb})

    nc = _build()
    res = run_bass_kernel_spmd(nc, in_maps, core_ids=list(range(NCORES)))
    LAST_RESULTS = res

    total = 0.0
    for r in res.results:
        total += float(r["loss_rows"].astype(np.float64).sum())
    return np.float32(total / N2)
